# revision 30
# baseline (speedup 1.0000x reference)
"""Trainium2 Bass kernel for nn_BlockSparseLocallyConnected.

Block-sparse locally-connected layer: 3x3 untied conv on a 32x32 grid,
32->32 channels, batch 128, expressed as 8836 dense 32x32 weight blocks
(BSR). Full inputs in, full output out; internally sharded over 8
NeuronCores by output tile-rows (weights are NOT replicated).

Decomposition: output space is covered by 16x16 spatial tiles of 2x2
positions. For output tile t, contributions come from its 4x4 input
window, which splits into four shifted 2x2 input blocks (passes
(a,b) in {0,1}^2). Each (tile, pass) is ONE tensor-engine matmul
  psum[(v,co), b] += lhsT[(u,ci), (v,co)].T @ rhs[(u,ci), b]
with K = 4 input positions x 32 cin = 128, M = 4 output positions x
32 cout = 128, N = batch = 128, accumulated over the 4 passes in PSUM.

Host pre-packs the input into "row-pair strips" xS[rp, (da,db,ci),
(j,b)] so that every matmul rhs is a contiguous SBUF slice of a strip
(no on-chip data rearrangement at all).

Weights are scattered host-side into padded [128,128] lhsT tiles in
the exact SBUF layout, so every lhsT is a contiguous slice (FWL-
eligible). Activations/weights/output ship as fp16 (PSUM accumulates
fp32); the kernel is DMA-bound, so halving the bytes dominates, and
fp16 keeps 10 mantissa bits (range is tiny here, so no overflow risk).
Weight chunks are staged across both HWDGE rings (SP + ACT) so the
weight stream is never starved behind the input strips; outputs drain
on the SWDGE ring. ~45 dummy matmuls warm the PE (HAM un-throttle to
2.4 GHz) while the first DMAs are in flight.
"""

import numpy as np

import concourse.bacc as bacc
import concourse.mybir as mybir
import concourse.tile as tile
from concourse.bass_utils import run_bass_kernel_spmd

# Problem constants (hardcoded; kernel.py must be self-contained).
B = 128          # batch
C = 32           # channels (in == out)
H = 32           # spatial height == width
NCORES = 8
NTJ = 16         # tile columns (W/2)
NTIL = 2         # tile rows per core (16 tile rows / 8 cores)
NSTRIP = 3       # row-pair strips per core
JSLOTS = 17      # j positions per strip (padded W/2 + 1)
SFREE = JSLOTS * B           # strip free dim = 2176
OCHUNK = 4                   # tj tiles per output DMA chunk
# weight chunk plan: (til, tj0, ntj, ring 0=ACT/1=SP)
CHPLAN = (
    (0, 0, 1, 0), (0, 1, 3, 0), (0, 4, 4, 0), (0, 8, 4, 0), (0, 12, 4, 1),
    (1, 0, 4, 0), (1, 4, 4, 1), (1, 8, 4, 0), (1, 12, 2, 1), (1, 14, 2, 0),
)
PASSES = ((0, 0), (0, 1), (1, 0), (1, 1))
F32 = mybir.dt.float32

DT = mybir.dt.float16
NPDT = np.float16
ODT = mybir.dt.float16

_NC_CACHE = {}


def _build_nc():
    """Build + compile the SPMD Bass module (one program, 8 cores)."""
    nc = bacc.Bacc(None, target_bir_lowering=False)

    xs_d = nc.dram_tensor("xs", [NSTRIP, 128, SFREE], DT, kind="ExternalInput")
    wt_d = nc.dram_tensor("wt", [NTIL, 128, NTJ * 4 * 128], DT, kind="ExternalInput")
    bias_d = nc.dram_tensor("bias", [128, NTIL * NTJ], F32, kind="ExternalInput")
    out_d = nc.dram_tensor("out", [NTIL, 128, NTJ * B], ODT, kind="ExternalOutput")

    with tile.TileContext(nc) as tc:
        with (
            tc.tile_pool(name="xpool", bufs=NSTRIP) as xpool,
            tc.tile_pool(name="bpool", bufs=1) as bpool,
            tc.tile_pool(name="wpool", bufs=1) as wpool,
            tc.tile_pool(name="opool", bufs=8) as opool,
            tc.tile_pool(name="psum", bufs=8, space="PSUM") as psum,
        ):
            # PE warm-up: ~45 dummy matmuls on a zeroed tile while the
            # DMAs stream in, so HAM un-throttles (1.2 -> 2.4 GHz)
            # before the first real matmul
            warm = bpool.tile([128, 128], DT, tag="warm")
            nc.vector.memset(warm[:], 0.0)
            wps = psum.tile([128, B], F32, tag="acc")
            for i in range(32):
                nc.tensor.matmul(wps[:], warm[:], warm[:], start=True, stop=True)

            # strips first on the SP ring (compute can't start without
            # them); weight chunks mostly on the ACT ring
            strips = []
            for s in range(NSTRIP):
                st = xpool.tile([128, SFREE], DT, tag="strip")
                nc.sync.dma_start(st[:], xs_d[s])
                strips.append(st)

            bias_t = bpool.tile([128, NTIL * NTJ], F32)
            nc.sync.dma_start(bias_t[:], bias_d[:])

            # padded weight chunks, alternating between the two HWDGE
            # rings so the weight stream gets the full DMA bandwidth
            # staged weight chunks: tiny first chunk so the PE starts
            # early, big chunks later; spread across both HWDGE rings
            chunk_of = {}
            for i, (til, tj0, ntj, eng) in enumerate(CHPLAN):
                wt_t = wpool.tile([128, ntj * 4 * 128], DT, tag=f"w{i}")
                (nc.scalar if eng == 0 else nc.sync).dma_start(
                    wt_t[:],
                    wt_d[til, :, tj0 * 4 * 128:(tj0 + ntj) * 4 * 128],
                )
                for tj in range(tj0, tj0 + ntj):
                    chunk_of[(til, tj)] = (wt_t, tj - tj0)

            for til in range(NTIL):
                for tj in range(NTJ):
                    if tj % OCHUNK == 0:
                        out_t = opool.tile([128, OCHUNK * B], ODT, tag="out")
                    ps = psum.tile([128, B], F32, tag="acc")
                    for pi, (a, b) in enumerate(PASSES):
                        rhs = strips[til + a][:, (tj + b) * B:(tj + b + 1) * B]
                        wt_t, rtj = chunk_of[(til, tj)]
                        lhsT = wt_t[:, (rtj * 4 + pi) * 128:(rtj * 4 + pi + 1) * 128]
                        nc.tensor.matmul(
                            ps[:], lhsT, rhs, start=(pi == 0), stop=(pi == 3)
                        )
                    # bias add + evacuate PSUM -> SBUF (alternate DVE /
                    # ACT so the final evacuations don't serialize)
                    oslice = out_t[:, (tj % OCHUNK) * B:(tj % OCHUNK + 1) * B]
                    bcol = bias_t[:, til * NTJ + tj:til * NTJ + tj + 1]
                    if tj % 2 == 0:
                        nc.vector.tensor_scalar_add(oslice, ps[:], bcol)
                    else:
                        nc.scalar.activation(
                            oslice, ps[:],
                            mybir.ActivationFunctionType.Identity, bias=bcol,
                        )
                    if tj % OCHUNK == OCHUNK - 1:
                        o0 = (tj // OCHUNK) * OCHUNK
                        nc.scalar.dma_start(
                            out_d[til, :, o0 * B:(o0 + OCHUNK) * B], out_t[:]
                        )

    nc.compile()
    return nc


def _pack_host(input, weight, mask, bias, brow_ids, bcol_ids):
    """Host-side packing of full inputs into per-core device arrays."""
    f32 = np.float32
    x = np.ascontiguousarray(np.asarray(input, dtype=f32))
    vals = np.asarray(weight, dtype=f32) * np.asarray(mask, dtype=f32)
    bias = np.asarray(bias, dtype=f32)
    p_sp = np.asarray(brow_ids).astype(np.int64)
    q_sp = np.asarray(bcol_ids).astype(np.int64)

    # --- input strips: xS[rp, (da,db,ci), (j,b)] = xpad[2rp+da, 2j+db, ci, b]
    x_t = np.transpose(x, (2, 3, 1, 0))                # [h, w, ci, b]
    xpad = np.zeros((H + 2, H + 2, C, B), f32)
    xpad[1:H + 1, 1:H + 1] = x_t
    xS = np.ascontiguousarray(
        xpad.reshape(JSLOTS, 2, JSLOTS, 2, C, B)
        .transpose(0, 1, 3, 4, 2, 5)
        .reshape(JSLOTS, 128, SFREE)
    ).astype(NPDT)

    # --- weights: scatter blocks into padded lhsT tiles, then slice the
    # valid slots into the flat [WROWS, tj, co] DMA stream
    ph, pw = p_sp // H, p_sp % H
    qh, qw = q_sp // H, q_sp % H
    ti, va = ph // 2, ph % 2
    tjc, vb = pw // 2, pw % 2
    ra = qh + 1 - 2 * ti          # = 2a + da in 0..3
    rb = qw + 1 - 2 * tjc         # = 2b + db in 0..3
    aa, da = ra // 2, ra % 2
    bb, db = rb // 2, rb % 2
    core, til = ti // 2, ti % 2
    mm = ((core * NTIL + til) * NTJ + tjc) * 4 + (aa * 2 + bb)
    u = da * 2 + db
    v = va * 2 + vb
    wflat = np.zeros((NCORES * NTIL * NTJ * 4, 4, C, 4, C), NPDT)  # [mm,u,ci,v,co]
    wflat[mm, u, :, v, :] = vals.transpose(0, 2, 1)
    # -> per-core SBUF layout [til, k=(u,ci), (tj, pass, m=(v,co))]
    w6 = wflat.reshape(NCORES, NTIL, NTJ, 4, 128, 128)
    w_cores = [
        np.ascontiguousarray(
            w6[c].transpose(0, 3, 1, 2, 4).reshape(NTIL, 128, NTJ * 4 * 128)
        )
        for c in range(NCORES)
    ]

    # --- bias: [ (va,vb,co), (til,tj) ] per core
    b3 = bias.reshape(H, H, C).reshape(NCORES, NTIL, 2, NTJ, 2, C)
    bias_cores = [
        np.ascontiguousarray(
            b3[c].transpose(1, 3, 4, 0, 2).reshape(128, NTIL * NTJ)
        )
        for c in range(NCORES)
    ]

    in_maps = []
    for c in range(NCORES):
        in_maps.append({
            "xs": np.ascontiguousarray(xS[2 * c:2 * c + NSTRIP]),
            "wt": w_cores[c],
            "bias": bias_cores[c],
        })
    return in_maps


def _unpack_host(results):
    """[c][til, (va,vb,co), (tj,b)] -> [b, co, h, w]"""
    out_all = np.stack([np.asarray(r["out"], dtype=np.float32) for r in results])
    o = out_all.reshape(NCORES, NTIL, 2, 2, C, NTJ, B)    # [c,til,va,vb,co,tj,b]
    o = o.transpose(6, 4, 0, 1, 2, 5, 3)                  # [b,co,c,til,va,tj,vb]
    return np.ascontiguousarray(o.reshape(B, C, H, H))


def kernel(input, weight, mask, bias, brow_ids, bcol_ids, _perf=None):
    if "nc" not in _NC_CACHE:
        _NC_CACHE["nc"] = _build_nc()
    nc = _NC_CACHE["nc"]
    in_maps = _pack_host(input, weight, mask, bias, brow_ids, bcol_ids)
    kwargs = dict(_perf) if _perf else {}
    res = run_bass_kernel_spmd(nc, in_maps, core_ids=list(range(NCORES)), **kwargs)
    if _perf is not None:
        _NC_CACHE["last_result"] = res
    return _unpack_host(res.results)


# revision 31
# speedup vs baseline: 1.0306x; 1.0306x over previous
"""Trainium2 Bass kernel for nn_BlockSparseLocallyConnected.

Block-sparse locally-connected layer: 3x3 untied conv on a 32x32 grid,
32->32 channels, batch 128, expressed as 8836 dense 32x32 weight blocks
(BSR). Full inputs in, full output out; internally sharded over 8
NeuronCores by output tile-rows (weights are NOT replicated).

Decomposition: output space is covered by 16x16 spatial tiles of 2x2
positions. For output tile t, contributions come from its 4x4 input
window, which splits into four shifted 2x2 input blocks (passes
(a,b) in {0,1}^2). Each (tile, pass) is ONE tensor-engine matmul
  psum[(v,co), b] += lhsT[(u,ci), (v,co)].T @ rhs[(u,ci), b]
with K = 4 input positions x 32 cin = 128, M = 4 output positions x
32 cout = 128, N = batch = 128, accumulated over the 4 passes in PSUM.

Host pre-packs the input into "row-pair strips" xS[rp, (da,db,ci),
(j,b)] so that every matmul rhs is a contiguous SBUF slice of a strip
(no on-chip data rearrangement at all).

Weights are scattered host-side into padded [128,128] lhsT tiles in
the exact SBUF layout, so every lhsT is a contiguous slice (FWL-
eligible). Activations/weights/output ship as fp16 (PSUM accumulates
fp32); the kernel is DMA-bound, so halving the bytes dominates, and
fp16 keeps 10 mantissa bits (range is tiny here, so no overflow risk).
Weight chunks are staged across both HWDGE rings (SP + ACT) so the
weight stream is never starved behind the input strips; outputs drain
on the SWDGE ring. ~45 dummy matmuls warm the PE (HAM un-throttle to
2.4 GHz) while the first DMAs are in flight.
"""

import numpy as np

import concourse.bacc as bacc
import concourse.mybir as mybir
import concourse.tile as tile
from concourse.bass_utils import run_bass_kernel_spmd

# Problem constants (hardcoded; kernel.py must be self-contained).
B = 128          # batch
C = 32           # channels (in == out)
H = 32           # spatial height == width
NCORES = 8
NTJ = 16         # tile columns (W/2)
NTIL = 2         # tile rows per core (16 tile rows / 8 cores)
NSTRIP = 3       # row-pair strips per core
JSLOTS = 17      # j positions per strip (padded W/2 + 1)
SFREE = JSLOTS * B           # strip free dim = 2176
OCHUNK = 4                   # tj tiles per output DMA chunk
# weight chunk plan: (til, tj0, ntj, ring 0=ACT/1=SP)
CHPLAN = (
    (0, 0, 1, 0), (0, 1, 3, 0), (0, 4, 4, 0), (0, 8, 4, 0), (0, 12, 4, 1),
    (1, 0, 4, 0), (1, 4, 4, 1), (1, 8, 4, 0), (1, 12, 2, 1), (1, 14, 2, 0),
)
PASSES = ((0, 0), (0, 1), (1, 0), (1, 1))
F32 = mybir.dt.float32

DT = mybir.dt.float16
NPDT = np.float16
ODT = mybir.dt.float16

_NC_CACHE = {}


def _build_nc():
    """Build + compile the SPMD Bass module (one program, 8 cores)."""
    nc = bacc.Bacc(None, target_bir_lowering=False)

    xs_d = nc.dram_tensor("xs", [NSTRIP, 128, SFREE], DT, kind="ExternalInput")
    wt_d = nc.dram_tensor("wt", [NTIL, 128, NTJ * 4 * 128], DT, kind="ExternalInput")
    bias_d = nc.dram_tensor("bias", [128, NTIL * NTJ], F32, kind="ExternalInput")
    out_d = nc.dram_tensor("out", [NTIL, 128, NTJ * B], ODT, kind="ExternalOutput")

    with tile.TileContext(nc) as tc:
        with (
            tc.tile_pool(name="xpool", bufs=NSTRIP) as xpool,
            tc.tile_pool(name="bpool", bufs=1) as bpool,
            tc.tile_pool(name="wpool", bufs=1) as wpool,
            tc.tile_pool(name="opool", bufs=8) as opool,
            tc.tile_pool(name="psum", bufs=8, space="PSUM") as psum,
        ):
            # PE warm-up: ~45 dummy matmuls on a zeroed tile while the
            # DMAs stream in, so HAM un-throttles (1.2 -> 2.4 GHz)
            # before the first real matmul
            warm = bpool.tile([128, 128], DT, tag="warm")
            nc.vector.memset(warm[:], 0.0)
            wps = psum.tile([128, B], F32, tag="acc")
            for i in range(45):
                nc.tensor.matmul(wps[:], warm[:], warm[:], start=True, stop=True)

            # strips first on the SP ring (compute can't start without
            # them); weight chunks mostly on the ACT ring
            strips = []
            for s in range(NSTRIP):
                st = xpool.tile([128, SFREE], DT, tag="strip")
                nc.sync.dma_start(st[:], xs_d[s])
                strips.append(st)

            bias_t = bpool.tile([128, NTIL * NTJ], F32)
            nc.sync.dma_start(bias_t[:], bias_d[:])

            # padded weight chunks, alternating between the two HWDGE
            # rings so the weight stream gets the full DMA bandwidth
            # staged weight chunks: tiny first chunk so the PE starts
            # early, big chunks later; spread across both HWDGE rings
            chunk_of = {}
            for i, (til, tj0, ntj, eng) in enumerate(CHPLAN):
                wt_t = wpool.tile([128, ntj * 4 * 128], DT, tag=f"w{i}")
                (nc.scalar if eng == 0 else nc.sync).dma_start(
                    wt_t[:],
                    wt_d[til, :, tj0 * 4 * 128:(tj0 + ntj) * 4 * 128],
                )
                for tj in range(tj0, tj0 + ntj):
                    chunk_of[(til, tj)] = (wt_t, tj - tj0)

            for til in range(NTIL):
                for tj in range(NTJ):
                    if tj % OCHUNK == 0:
                        out_t = opool.tile([128, OCHUNK * B], ODT, tag="out")
                    ps = psum.tile([128, B], F32, tag="acc")
                    for pi, (a, b) in enumerate(PASSES):
                        rhs = strips[til + a][:, (tj + b) * B:(tj + b + 1) * B]
                        wt_t, rtj = chunk_of[(til, tj)]
                        lhsT = wt_t[:, (rtj * 4 + pi) * 128:(rtj * 4 + pi + 1) * 128]
                        nc.tensor.matmul(
                            ps[:], lhsT, rhs, start=(pi == 0), stop=(pi == 3)
                        )
                    # bias add + evacuate PSUM -> SBUF
                    nc.vector.tensor_scalar_add(
                        out_t[:, (tj % OCHUNK) * B:(tj % OCHUNK + 1) * B],
                        ps[:],
                        bias_t[:, til * NTJ + tj:til * NTJ + tj + 1],
                    )
                    if tj % OCHUNK == OCHUNK - 1:
                        o0 = (tj // OCHUNK) * OCHUNK
                        nc.scalar.dma_start(
                            out_d[til, :, o0 * B:(o0 + OCHUNK) * B], out_t[:]
                        )

    nc.compile()
    return nc


def _pack_host(input, weight, mask, bias, brow_ids, bcol_ids):
    """Host-side packing of full inputs into per-core device arrays."""
    f32 = np.float32
    x = np.ascontiguousarray(np.asarray(input, dtype=f32))
    vals = np.asarray(weight, dtype=f32) * np.asarray(mask, dtype=f32)
    bias = np.asarray(bias, dtype=f32)
    p_sp = np.asarray(brow_ids).astype(np.int64)
    q_sp = np.asarray(bcol_ids).astype(np.int64)

    # --- input strips: xS[rp, (da,db,ci), (j,b)] = xpad[2rp+da, 2j+db, ci, b]
    x_t = np.transpose(x, (2, 3, 1, 0))                # [h, w, ci, b]
    xpad = np.zeros((H + 2, H + 2, C, B), f32)
    xpad[1:H + 1, 1:H + 1] = x_t
    xS = np.ascontiguousarray(
        xpad.reshape(JSLOTS, 2, JSLOTS, 2, C, B)
        .transpose(0, 1, 3, 4, 2, 5)
        .reshape(JSLOTS, 128, SFREE)
    ).astype(NPDT)

    # --- weights: scatter blocks into padded lhsT tiles, then slice the
    # valid slots into the flat [WROWS, tj, co] DMA stream
    ph, pw = p_sp // H, p_sp % H
    qh, qw = q_sp // H, q_sp % H
    ti, va = ph // 2, ph % 2
    tjc, vb = pw // 2, pw % 2
    ra = qh + 1 - 2 * ti          # = 2a + da in 0..3
    rb = qw + 1 - 2 * tjc         # = 2b + db in 0..3
    aa, da = ra // 2, ra % 2
    bb, db = rb // 2, rb % 2
    core, til = ti // 2, ti % 2
    mm = ((core * NTIL + til) * NTJ + tjc) * 4 + (aa * 2 + bb)
    u = da * 2 + db
    v = va * 2 + vb
    wflat = np.zeros((NCORES * NTIL * NTJ * 4, 4, C, 4, C), NPDT)  # [mm,u,ci,v,co]
    wflat[mm, u, :, v, :] = vals.transpose(0, 2, 1)
    # -> per-core SBUF layout [til, k=(u,ci), (tj, pass, m=(v,co))]
    w6 = wflat.reshape(NCORES, NTIL, NTJ, 4, 128, 128)
    w_cores = [
        np.ascontiguousarray(
            w6[c].transpose(0, 3, 1, 2, 4).reshape(NTIL, 128, NTJ * 4 * 128)
        )
        for c in range(NCORES)
    ]

    # --- bias: [ (va,vb,co), (til,tj) ] per core
    b3 = bias.reshape(H, H, C).reshape(NCORES, NTIL, 2, NTJ, 2, C)
    bias_cores = [
        np.ascontiguousarray(
            b3[c].transpose(1, 3, 4, 0, 2).reshape(128, NTIL * NTJ)
        )
        for c in range(NCORES)
    ]

    in_maps = []
    for c in range(NCORES):
        in_maps.append({
            "xs": np.ascontiguousarray(xS[2 * c:2 * c + NSTRIP]),
            "wt": w_cores[c],
            "bias": bias_cores[c],
        })
    return in_maps


def _unpack_host(results):
    """[c][til, (va,vb,co), (tj,b)] -> [b, co, h, w]"""
    out_all = np.stack([np.asarray(r["out"], dtype=np.float32) for r in results])
    o = out_all.reshape(NCORES, NTIL, 2, 2, C, NTJ, B)    # [c,til,va,vb,co,tj,b]
    o = o.transpose(6, 4, 0, 1, 2, 5, 3)                  # [b,co,c,til,va,tj,vb]
    return np.ascontiguousarray(o.reshape(B, C, H, H))


def kernel(input, weight, mask, bias, brow_ids, bcol_ids, _perf=None):
    if "nc" not in _NC_CACHE:
        _NC_CACHE["nc"] = _build_nc()
    nc = _NC_CACHE["nc"]
    in_maps = _pack_host(input, weight, mask, bias, brow_ids, bcol_ids)
    kwargs = dict(_perf) if _perf else {}
    res = run_bass_kernel_spmd(nc, in_maps, core_ids=list(range(NCORES)), **kwargs)
    if _perf is not None:
        _NC_CACHE["last_result"] = res
    return _unpack_host(res.results)
